# revision 13
# baseline (speedup 1.0000x reference)
"""Trainium2 Bass kernel for nn_AttentionBlock (B=4, S=2048, H=1024, NH=16, FFN=4096).

Sharding: 8 cores = 4 batches x 2 query-halves. Each core computes k/v for its
batch's full sequence (recompute, zero cross-core communication) and the full
block output for its 1024 query tokens. Tokens are host-permuted so each core's
query tokens are always [0:1024) - the SPMD program is offset-uniform.

kernel(**inputs) -> np.ndarray takes FULL inputs, shards on host, runs the SPMD
NEFF on cores 0-7, reassembles the full (4, 2048, 1024) output.
"""
import sys

sys.path.insert(0, "/opt/trn_rl_repo")

from contextlib import ExitStack

import numpy as np
import ml_dtypes

import concourse.bass as bass
import concourse.bacc as bacc
import concourse.tile as tile
import concourse.mybir as mybir
from concourse.bass_utils import run_bass_kernel_spmd

dt = mybir.dt
Alu = mybir.AluOpType
Act = mybir.ActivationFunctionType

B, S, H, NH, DK, FFN = 4, 2048, 1024, 16, 64, 4096
Q = S // 2          # query tokens per core
SCALE = DK ** -0.5
HC = H // 128       # 8 h-chunks
SC = S // 128       # 16 token chunks (kv)
FC = FFN // 128     # 32 ffn chunks
EPS = 1e-5

_CACHE = {}


def build_nc():
    nc = bacc.Bacc("TRN2", target_bir_lowering=False, debug=False, num_devices=8)

    xT = nc.dram_tensor("xT", [H, S], dt.float32, kind="ExternalInput").ap()
    biasT = nc.dram_tensor("biasT", [NH, S, Q], dt.float32, kind="ExternalInput").ap()
    wq = nc.dram_tensor("wq", [H, H], dt.float32, kind="ExternalInput").ap()
    wk = nc.dram_tensor("wk", [H, H], dt.float32, kind="ExternalInput").ap()
    wv = nc.dram_tensor("wv", [H, H], dt.float32, kind="ExternalInput").ap()
    wo = nc.dram_tensor("wo", [H, H], dt.float32, kind="ExternalInput").ap()
    # w1t[g] = W1_eff[:, g*512:(g+1)*512]  (f-groups of 512)
    w1t = nc.dram_tensor("w1t", [FFN // 512, H, 512], dt.float32, kind="ExternalInput").ap()
    # w2t[jc] = W2[:, jc*128:(jc+1)*128] in bf16
    w2t = nc.dram_tensor("w2t", [HC, FFN, 128], dt.bfloat16, kind="ExternalInput").ap()
    wsums = nc.dram_tensor("wsums", [3, H], dt.float32, kind="ExternalInput").ap()
    b1c = nc.dram_tensor("b1c", [128, FC], dt.float32, kind="ExternalInput").ap()
    boc = nc.dram_tensor("boc", [128, HC], dt.float32, kind="ExternalInput").ap()
    b2c = nc.dram_tensor("b2c", [128, HC], dt.float32, kind="ExternalInput").ap()

    outT = nc.dram_tensor("outT", [H, Q], dt.float32, kind="ExternalOutput").ap()
    import os
    KDEBUG = bool(int(os.environ.get("KDEBUG", "0")))
    if KDEBUG:
        dbg_rstd = nc.dram_tensor("dbg_rstd", [1, S], dt.float32, kind="ExternalOutput").ap()
        dbg_negm = nc.dram_tensor("dbg_negm", [1, S], dt.float32, kind="ExternalOutput").ap()
        dbg_q = nc.dram_tensor("dbg_q", [128, Q], dt.float32, kind="ExternalOutput").ap()
        dbg_k = nc.dram_tensor("dbg_k", [128, S], dt.float32, kind="ExternalOutput").ap()
        dbg_v = nc.dram_tensor("dbg_v", [128, NH * 65], dt.float32, kind="ExternalOutput").ap()
        dbg_ao = nc.dram_tensor("dbg_ao", [128, Q], dt.float32, kind="ExternalOutput").ap()
        dbg_x2 = nc.dram_tensor("dbg_x2", [128, Q], dt.float32, kind="ExternalOutput").ap()
        dbg_x2n = nc.dram_tensor("dbg_x2n", [128, Q], dt.float32, kind="ExternalOutput").ap()

    with tile.TileContext(nc) as tc, ExitStack() as ctx:
        smalls = ctx.enter_context(tc.tile_pool(name="smalls", bufs=1))
        ao_pool = ctx.enter_context(tc.tile_pool(name="ao_pool", bufs=1))

        aoT = [ao_pool.tile([128, Q], dt.float32r, name=f"aoT{c}") for c in range(HC)]

        ones_bf = smalls.tile([128, 1], dt.bfloat16, name="ones_bf")
        nc.vector.memset(ones_bf[:], 1.0)
        eps_t = smalls.tile([1, 1], dt.float32, name="eps_t")
        nc.vector.memset(eps_t[:], EPS)
        negm = smalls.tile([1, S], dt.bfloat16, name="negm")
        rstd_row = smalls.tile([1, S], dt.float32, name="rstd_row")
        rb = smalls.tile([128, S], dt.float32, name="rb")
        rstd_col = smalls.tile([128, SC], dt.float32, name="rstd_col")
        wsum_bf = [smalls.tile([1, H], dt.bfloat16, name=f"wsum_bf{i}") for i in range(3)]
        for i in range(3):
            nc.gpsimd.dma_start(wsum_bf[i][:], wsums[i:i + 1, :])

        with (
            tc.tile_pool(name="qpool", bufs=1) as qpool,
            tc.tile_pool(name="kpool", bufs=1) as kpool,
            tc.tile_pool(name="vpool", bufs=1) as vpool,
        ):
            qT = [qpool.tile([128, Q], dt.bfloat16, name=f"qT{c}") for c in range(HC)]
            kT = [kpool.tile([128, S], dt.bfloat16, name=f"kT{c}") for c in range(HC)]
            vS = [vpool.tile([128, NH * 65], dt.bfloat16, name=f"vS{c}") for c in range(SC)]

            # ================ Phase A: LN1 stats + QKV (bf16) ================
            with (
                tc.tile_pool(name="xbf_pool", bufs=1) as xbf_pool,
                tc.tile_pool(name="sq_pool", bufs=2) as sq_pool,
                tc.tile_pool(name="wbf_pool", bufs=9) as wbf_pool,
                tc.tile_pool(name="stat_sb", bufs=1) as stat_sb,
                tc.tile_pool(name="pj", bufs=4, space="PSUM") as pj,
                tc.tile_pool(name="pstat", bufs=2, space="PSUM") as pstat,
            ):
                xbf = [xbf_pool.tile([128, S], dt.bfloat16, name=f"xbf{c}")
                       for c in range(HC)]
                for c in range(HC):
                    nc.gpsimd.dma_start(xbf[c][:], xT[c * 128:(c + 1) * 128, :])

                for tg in range(S // 512):
                    tsl = slice(tg * 512, (tg + 1) * 512)
                    psx = pstat.tile([1, 512], dt.float32, name="psx", tag="psx")
                    pss = pstat.tile([1, 512], dt.float32, name="pss", tag="pss")
                    for c in range(HC):
                        sq = sq_pool.tile([128, 512], dt.bfloat16, name="sq", tag="sq")
                        nc.vector.tensor_mul(sq[:], xbf[c][:, tsl], xbf[c][:, tsl])
                        nc.tensor.matmul(psx[:], ones_bf[:], xbf[c][:, tsl],
                                         start=(c == 0), stop=(c == HC - 1))
                        nc.tensor.matmul(pss[:], ones_bf[:], sq[:],
                                         start=(c == 0), stop=(c == HC - 1))
                    nc.vector.tensor_scalar_mul(negm[0:1, tsl], psx[:], -1.0 / H)
                    msq = stat_sb.tile([1, 512], dt.float32, name="msq", tag="msq")
                    nc.vector.tensor_mul(msq[:], negm[0:1, tsl], negm[0:1, tsl])
                    var = stat_sb.tile([1, 512], dt.float32, name="var", tag="var")
                    nc.vector.scalar_tensor_tensor(var[:], pss[:], 1.0 / H, msq[:],
                                                   op0=Alu.mult, op1=Alu.subtract)
                    lnv = stat_sb.tile([1, 512], dt.float32, name="lnv", tag="lnv")
                    nc.scalar.activation(lnv[:], var[:], Act.Ln, bias=eps_t[:])
                    nc.scalar.activation(rstd_row[0:1, tsl], lnv[:], Act.Exp, scale=-0.5)
                nc.gpsimd.partition_broadcast(rb[:], rstd_row[:])
                # SBUF free-dim -> partition-dim reshuffle must round-trip DRAM
                with tc.tile_pool(name="drs", bufs=1, space="DRAM") as drs:
                    r_dr = drs.tile([1, S], dt.float32, name="r_dr")
                    nc.sync.dma_start(r_dr[:], rstd_row[:])
                    nc.sync.dma_start(
                        rstd_col[:], r_dr.rearrange("x (c p) -> (x p) c", p=128))
                    negm_col = stat_sb.tile([128, SC], dt.float32, name="negm_col",
                                            tag="negm_col", bufs=1)
                    nm_dr = drs.tile([1, S], dt.float32, name="nm_dr")
                    nc.gpsimd.dma_start(nm_dr[:], negm[:])
                    nc.sync.dma_start(
                        negm_col[:], nm_dr.rearrange("x (c p) -> (x p) c", p=128))
                mrcol = stat_sb.tile([128, SC], dt.float32, name="mrcol",
                                     tag="mrcol", bufs=1)
                nc.vector.tensor_mul(mrcol[:], negm_col[:], rstd_col[:])
                wvs_row = stat_sb.tile([1, H], dt.float32, name="wvs_row",
                                       tag="wvs_row", bufs=1)
                nc.sync.dma_start(wvs_row[:], wsums[2:3, :])
                wvs_b = stat_sb.tile([128, H], dt.float32, name="wvs_b",
                                     tag="wvs_b", bufs=1)
                nc.gpsimd.partition_broadcast(wvs_b[:], wvs_row[:])

                for ip, (wdram, wbname) in enumerate([(wq, "wqb"), (wk, "wkb")]):
                    wb = []
                    for c in range(HC):
                        t = wbf_pool.tile([128, H], dt.bfloat16, name=f"{wbname}{c}",
                                          tag="wb")
                        nc.gpsimd.dma_start(t[:], wdram[c * 128:(c + 1) * 128, :])
                        wb.append(t)
                    ntok = Q if ip == 0 else S
                    dest = qT if ip == 0 else kT
                    for dc in range(HC):
                        for tg in range(ntok // 512):
                            tsl = slice(tg * 512, (tg + 1) * 512)
                            ps = pj.tile([128, 512], dt.float32, name="pqk", tag="pj")
                            for c in range(HC):
                                nc.tensor.matmul(ps[:], wb[c][:, dc * 128:(dc + 1) * 128],
                                                 xbf[c][:, tsl],
                                                 start=(c == 0), stop=False)
                            nc.tensor.matmul(
                                ps[:], wsum_bf[ip][0:1, dc * 128:(dc + 1) * 128],
                                negm[0:1, tsl], start=False, stop=True)
                            nc.vector.tensor_mul(dest[dc][:, tsl], ps[:], rb[:, tsl])

                wvb = []
                for c in range(HC):
                    t = wbf_pool.tile([128, H], dt.bfloat16, name=f"wvb{c}", tag="wb")
                    nc.gpsimd.dma_start(t[:], wv[c * 128:(c + 1) * 128, :])
                    wvb.append(t)
                for tci in range(SC):
                    vre = vS[tci].rearrange("p (h c) -> p h c", c=65)
                    nc.vector.memset(vre[:, :, 64:65], 1.0)
                    for dg in range(2):
                        dsl = slice(dg * 512, (dg + 1) * 512)
                        ps = pj.tile([128, 512], dt.float32, name="pv", tag="pj")
                        for c in range(HC):
                            nc.tensor.matmul(
                                ps[:], xbf[c][:, tci * 128:(tci + 1) * 128],
                                wvb[c][:, dsl], start=(c == 0), stop=(c == HC - 1))
                        # corr[tok, d] = (-m[tok] * rstd[tok]) * wvsum[d]
                        corrt = sq_pool.tile([128, 512], dt.bfloat16, name="corrt",
                                             tag="corrt", bufs=2)
                        nc.vector.tensor_scalar_mul(corrt[:], wvs_b[:, dsl],
                                                    mrcol[:, tci:tci + 1])
                        nc.vector.scalar_tensor_tensor(
                            vre[:, dg * 8:(dg + 1) * 8, 0:64],
                            ps[:].rearrange("p (h d) -> p h d", d=64),
                            rstd_col[:, tci:tci + 1],
                            corrt[:].rearrange("p (h d) -> p h d", d=64),
                            op0=Alu.mult, op1=Alu.add)

            if KDEBUG:
                with tc.tile_pool(name="dbgp", bufs=1) as dbgp:
                    dnm = dbgp.tile([1, S], dt.float32, name="dnm")
                    nc.vector.tensor_copy(dnm[:], negm[:])
                    nc.sync.dma_start(dbg_negm[:, :], dnm[:])
                    nc.sync.dma_start(dbg_rstd[:, :], rstd_row[:])
                    dq = dbgp.tile([128, Q], dt.float32, name="dq")
                    nc.vector.tensor_copy(dq[:], qT[0][:])
                    nc.sync.dma_start(dbg_q[:, :], dq[:])
                    dk = dbgp.tile([128, S], dt.float32, name="dk")
                    nc.vector.tensor_copy(dk[:], kT[0][:])
                    nc.sync.dma_start(dbg_k[:, :], dk[:])
                    dv = dbgp.tile([128, NH * 65], dt.float32, name="dv")
                    nc.vector.tensor_copy(dv[:], vS[0][:])
                    nc.sync.dma_start(dbg_v[:, :], dv[:])

            tc.strict_bb_all_engine_barrier()
            # ================ Phase B: attention (bf16) ================
            with (
                tc.tile_pool(name="e_pool", bufs=4) as e_pool,
                tc.tile_pool(name="bias_pool", bufs=3) as bias_pool,
                tc.tile_pool(name="nrm_pool", bufs=1) as nrm_pool,
                tc.tile_pool(name="psc", bufs=2, space="PSUM") as psc,
                tc.tile_pool(name="pao", bufs=2, space="PSUM") as pao,
            ):
                for hp in range(NH // 2):
                    hc = hp
                    ao_ps = [pao.tile([65, Q], dt.float32, name=f"ao{h}", tag="ao")
                             for h in range(2)]
                    for kc in range(SC):
                        vre = vS[kc].rearrange("p (h c) -> p h c", c=65)
                        for h in range(2):
                            po = h * 64
                            scp = psc.tile([128, Q], dt.float32, name="scp", tag="sc")
                            for qg in range(2):
                                qsl = slice(qg * 512, (qg + 1) * 512)
                                nc.tensor.matmul(
                                    scp[:, qsl],
                                    kT[hc][po:po + 64, kc * 128:(kc + 1) * 128],
                                    qT[hc][po:po + 64, qsl],
                                    start=True, stop=True)
                            bt = bias_pool.tile([128, Q], dt.float32, name="bt", tag="bt")
                            nc.sync.dma_start(
                                bt[:], biasT[2 * hp + h, kc * 128:(kc + 1) * 128, :])
                            nc.vector.tensor_add(scp[:], scp[:], bt[:])
                            et = e_pool.tile([128, Q], dt.bfloat16, name="et", tag="et")
                            nc.scalar.activation(et[:], scp[:], Act.Exp)
                            v65 = vre[:, 2 * hp + h, :]
                            for qg in range(2):
                                qsl = slice(qg * 512, (qg + 1) * 512)
                                nc.tensor.matmul(ao_ps[h][:, qsl], v65, et[:, qsl],
                                                 start=(kc == 0), stop=(kc == SC - 1))
                    for h in range(2):
                        recip = nrm_pool.tile([1, Q], dt.float32, name="recip",
                                              tag="recip", bufs=1)
                        nc.vector.reciprocal(recip[:], ao_ps[h][64:65, :])
                        rb64 = nrm_pool.tile([64, Q], dt.float32, name="rb64",
                                             tag="rb64", bufs=1)
                        nc.gpsimd.partition_broadcast(rb64[:], recip[:])
                        if h == 0:
                            nc.vector.tensor_mul(aoT[hc][0:64, :], ao_ps[h][0:64, :],
                                                 rb64[:])
                        else:
                            # DVE cannot shift partitions: normalize at base 0,
                            # then DMA the 64 rows into partitions 64-127.
                            t64 = nrm_pool.tile([64, Q], dt.float32r, name="t64",
                                                tag="t64", bufs=1)
                            nc.vector.tensor_mul(t64[:], ao_ps[h][0:64, :], rb64[:])
                            nc.sync.dma_start(aoT[hc][64:128, :], t64[:])

        tc.strict_bb_all_engine_barrier()
        # ================ Phase C: Wo + residual, LN2, FFN ================
        x2_pool = ctx.enter_context(tc.tile_pool(name="x2_pool", bufs=1))
        x2T = [x2_pool.tile([128, Q], dt.float32, name=f"x2T{c}") for c in range(HC)]
        with (
            tc.tile_pool(name="wo_raw", bufs=2) as wo_raw,
            tc.tile_pool(name="wo_pool", bufs=8) as wo_pool,
            tc.tile_pool(name="xq_pool", bufs=1) as xq_pool,
            tc.tile_pool(name="bvec", bufs=1) as bvec,
            tc.tile_pool(name="pwo", bufs=4, space="PSUM") as pwo,
        ):
            bo_sb = bvec.tile([128, HC], dt.float32, name="bo_sb")
            nc.sync.dma_start(bo_sb[:], boc[:, :])
            wof = []
            for c in range(HC):
                raw = wo_raw.tile([128, H], dt.float32, name="wor", tag="wor")
                nc.sync.dma_start(raw[:], wo[c * 128:(c + 1) * 128, :])
                t = wo_pool.tile([128, H], dt.float32r, name=f"wof{c}", tag="wof")
                nc.gpsimd.tensor_copy(t[:], raw[:])
                wof.append(t)
            xq = [xq_pool.tile([128, Q], dt.float32, name=f"xq{c}") for c in range(HC)]
            for c in range(HC):
                nc.sync.dma_start(xq[c][:], xT[c * 128:(c + 1) * 128, 0:Q])
            for jc in range(HC):
                for tg in range(2):
                    tsl = slice(tg * 512, (tg + 1) * 512)
                    ps = pwo.tile([128, 512], dt.float32, name="pwo_t", tag="pwo_t")
                    for c in range(HC):
                        nc.tensor.matmul(ps[:], wof[c][:, jc * 128:(jc + 1) * 128],
                                         aoT[c][:, tsl],
                                         start=(c == 0), stop=(c == HC - 1))
                    nc.vector.scalar_tensor_tensor(
                        x2T[jc][:, tsl], ps[:], bo_sb[:, jc:jc + 1], xq[jc][:, tsl],
                        op0=Alu.add, op1=Alu.add)

        if KDEBUG:
            with tc.tile_pool(name="dbgp2", bufs=1) as dbgp2:
                da = dbgp2.tile([128, Q], dt.float32, name="da")
                nc.vector.tensor_copy(da[:], aoT[0][:])
                nc.sync.dma_start(dbg_ao[:, :], da[:])
                nc.sync.dma_start(dbg_x2[:, :], x2T[0][:])

        # LN2 stats (bf16 copies for partition-sum matmuls)
        mstat = ctx.enter_context(tc.tile_pool(name="mstat", bufs=1))
        m2 = mstat.tile([1, Q], dt.float32, name="m2")
        m2b = mstat.tile([128, Q], dt.float32, name="m2b")
        r2b = mstat.tile([128, Q], dt.float32, name="r2b")
        with (
            tc.tile_pool(name="ln2_sb", bufs=2) as ln2_sb,
            tc.tile_pool(name="pstat2", bufs=2, space="PSUM") as pstat2,
        ):
            r2row = ln2_sb.tile([1, Q], dt.float32, name="r2row", tag="r2row", bufs=1)
            for tg in range(Q // 512):
                tsl = slice(tg * 512, (tg + 1) * 512)
                psx = pstat2.tile([1, 512], dt.float32, name="psx2", tag="psx2")
                pss = pstat2.tile([1, 512], dt.float32, name="pss2", tag="pss2")
                for c in range(HC):
                    xb2 = ln2_sb.tile([128, 512], dt.bfloat16, name="xb2", tag="xb2")
                    nc.vector.tensor_copy(xb2[:], x2T[c][:, tsl])
                    sq2 = ln2_sb.tile([128, 512], dt.bfloat16, name="sq2", tag="sq2")
                    nc.vector.tensor_mul(sq2[:], xb2[:], xb2[:])
                    nc.tensor.matmul(psx[:], ones_bf[:], xb2[:],
                                     start=(c == 0), stop=(c == HC - 1))
                    nc.tensor.matmul(pss[:], ones_bf[:], sq2[:],
                                     start=(c == 0), stop=(c == HC - 1))
                nc.vector.tensor_scalar_mul(m2[0:1, tsl], psx[:], 1.0 / H)
                msq2 = ln2_sb.tile([1, 512], dt.float32, name="msq2", tag="msq2")
                nc.vector.tensor_mul(msq2[:], m2[0:1, tsl], m2[0:1, tsl])
                var2 = ln2_sb.tile([1, 512], dt.float32, name="var2", tag="var2")
                nc.vector.scalar_tensor_tensor(var2[:], pss[:], 1.0 / H, msq2[:],
                                               op0=Alu.mult, op1=Alu.subtract)
                lnv2 = ln2_sb.tile([1, 512], dt.float32, name="lnv2", tag="lnv2")
                nc.scalar.activation(lnv2[:], var2[:], Act.Ln, bias=eps_t[:])
                nc.scalar.activation(r2row[0:1, tsl], lnv2[:], Act.Exp, scale=-0.5)
            nc.gpsimd.partition_broadcast(m2b[:], m2[:])
            nc.gpsimd.partition_broadcast(r2b[:], r2row[:])

        tc.strict_bb_all_engine_barrier()
        # x2n = (x2 - m2) * r2  (bf16, feeds W1); FFN in two token-halves
        with (
            tc.tile_pool(name="x2n_pool", bufs=1) as x2n_pool,
            tc.tile_pool(name="h2_pool", bufs=1) as h2_pool,
            tc.tile_pool(name="w1_pool", bufs=8) as w1_pool,
            tc.tile_pool(name="w2_pool", bufs=4) as w2_pool,
            tc.tile_pool(name="fvec", bufs=1) as fvec,
            tc.tile_pool(name="out_pool", bufs=1) as out_pool,
            tc.tile_pool(name="tmpn", bufs=1) as tmpn,
            tc.tile_pool(name="pw1", bufs=2, space="PSUM") as pw1,
            tc.tile_pool(name="pw2", bufs=2, space="PSUM") as pw2,
        ):
            x2n = [x2n_pool.tile([128, Q], dt.bfloat16, name=f"x2n{c}")
                   for c in range(HC)]
            for c in range(HC):
                tmp = tmpn.tile([128, Q], dt.float32, name="x2tmp", tag="x2tmp")
                nc.vector.tensor_sub(tmp[:], x2T[c][:], m2b[:])
                nc.vector.tensor_mul(x2n[c][:], tmp[:], r2b[:])

            if KDEBUG:
                dxn = tmpn.tile([128, Q], dt.float32, name="dxn", tag="x2tmp")
                nc.vector.tensor_copy(dxn[:], x2n[0][:])
                nc.sync.dma_start(dbg_x2n[:, :], dxn[:])

            b1_sb = fvec.tile([128, FC], dt.float32, name="b1_sb")
            nc.sync.dma_start(b1_sb[:], b1c[:, :])
            b2_sb = fvec.tile([128, HC], dt.float32, name="b2_sb")
            nc.sync.dma_start(b2_sb[:], b2c[:, :])

            for th in range(2):
                hsl = slice(th * 512, (th + 1) * 512)
                h2 = [h2_pool.tile([128, 512], dt.bfloat16, name=f"h2_{f}",
                                   tag=f"h2_{f}") for f in range(FC)]
                for fg in range(FFN // 512):
                    w1f = []
                    for c in range(HC):
                        t = w1_pool.tile([128, 512], dt.bfloat16, name="w1f", tag="w1f")
                        nc.gpsimd.dma_start(t[:], w1t[fg, c * 128:(c + 1) * 128, :])
                        w1f.append(t)
                    for fs in range(4):
                        ft = fg * 4 + fs
                        ps = pw1.tile([128, 512], dt.float32, name="pw1_t", tag="pw1_t")
                        for c in range(HC):
                            nc.tensor.matmul(
                                ps[:], w1f[c][:, fs * 128:(fs + 1) * 128],
                                x2n[c][:, hsl], start=(c == 0), stop=(c == HC - 1))
                        nc.scalar.activation(h2[ft][:], ps[:], Act.Gelu,
                                             bias=b1_sb[:, ft:ft + 1])

                for jc in range(HC):
                    w2f = []
                    for fg in range(FFN // 512):
                        t = w2_pool.tile([128, 4, 128], dt.bfloat16, name="w2f",
                                         tag="w2f")
                        nc.sync.dma_start(
                            t[:], w2t[jc, fg * 512:(fg + 1) * 512, :]
                            .rearrange("(c p) j -> p c j", p=128))
                        w2f.append(t)
                    pso = pw2.tile([128, 512], dt.float32, name="pso", tag="pso")
                    for fc in range(FC):
                        nc.tensor.matmul(pso[:], w2f[fc // 4][:, fc % 4, :],
                                         h2[fc][:],
                                         start=(fc == 0), stop=(fc == FC - 1))
                    ott = out_pool.tile([128, 512], dt.float32, name="ott", tag="ott",
                                        bufs=2)
                    nc.vector.scalar_tensor_tensor(
                        ott[:], pso[:], b2_sb[:, jc:jc + 1], x2T[jc][:, hsl],
                        op0=Alu.add, op1=Alu.add)
                    nc.sync.dma_start(outT[jc * 128:(jc + 1) * 128, hsl], ott[:])

    nc.compile()
    return nc


def _prep_inputs(x, attn_bias, ln1_g, ln1_b, Wq, bq, Wk, bk, Wv, bv, Wo, bo,
                 ln2_g, ln2_b, W1, b1, W2, b2):
    f32 = np.float32
    x = np.asarray(x, f32)
    attn_bias = np.asarray(attn_bias, f32)
    wq_e = (np.asarray(ln1_g, f32)[:, None] * np.asarray(Wq, f32)) * SCALE
    wk_e = np.asarray(ln1_g, f32)[:, None] * np.asarray(Wk, f32)
    wv_e = np.asarray(ln1_g, f32)[:, None] * np.asarray(Wv, f32)
    bq_e = (np.asarray(bq, f32) + np.asarray(ln1_b, f32) @ np.asarray(Wq, f32)) * SCALE
    bk_e = np.asarray(bk, f32) + np.asarray(ln1_b, f32) @ np.asarray(Wk, f32)
    bv_e = np.asarray(bv, f32) + np.asarray(ln1_b, f32) @ np.asarray(Wv, f32)
    assert np.abs(bq_e).max() == 0 and np.abs(bk_e).max() == 0 and np.abs(bv_e).max() == 0, \
        "nonzero qkv biases not supported by this build"
    w1_e = np.asarray(ln2_g, f32)[:, None] * np.asarray(W1, f32)
    b1_e = np.asarray(b1, f32) + np.asarray(ln2_b, f32) @ np.asarray(W1, f32)

    wsums = np.stack([wq_e.sum(0), wk_e.sum(0), wv_e.sum(0)]).astype(f32)
    w1t = np.ascontiguousarray(
        w1_e.reshape(H, FFN // 512, 512).transpose(1, 0, 2)).astype(f32)
    w2t = np.ascontiguousarray(
        np.asarray(W2, f32).reshape(FFN, HC, 128).transpose(1, 0, 2)
    ).astype(ml_dtypes.bfloat16)
    b1cc = np.ascontiguousarray(b1_e.reshape(FC, 128).T).astype(f32)
    bocc = np.ascontiguousarray(np.asarray(bo, f32).reshape(HC, 128).T).astype(f32)
    b2cc = np.ascontiguousarray(np.asarray(b2, f32).reshape(HC, 128).T).astype(f32)

    shared = dict(wq=wq_e, wk=wk_e, wv=wv_e, wo=np.asarray(Wo, f32),
                  w1t=w1t, w2t=w2t, wsums=wsums, b1c=b1cc, boc=bocc, b2c=b2cc)

    in_maps = []
    for c in range(8):
        b_i, qh = c // 2, c % 2
        perm = np.concatenate([np.arange(qh * Q, (qh + 1) * Q),
                               np.arange((1 - qh) * Q, (2 - qh) * Q)])
        xT_p = np.ascontiguousarray(x[b_i][perm].T)
        bT = np.ascontiguousarray(
            attn_bias[0][:, qh * Q:(qh + 1) * Q, :][:, :, perm].transpose(0, 2, 1))
        in_maps.append(dict(shared, xT=xT_p, biasT=bT))
    return in_maps


def kernel(**inputs) -> np.ndarray:
    if "nc" not in _CACHE:
        _CACHE["nc"] = build_nc()
    nc = _CACHE["nc"]
    in_maps = _prep_inputs(**inputs)
    res = run_bass_kernel_spmd(nc, in_maps, core_ids=list(range(8)))
    out = np.empty((B, S, H), np.float32)
    for c in range(8):
        b_i, qh = c // 2, c % 2
        out[b_i, qh * Q:(qh + 1) * Q, :] = res.results[c]["outT"].T
    return out


if __name__ == "__main__":
    import importlib
    ref = importlib.import_module("reference")
    ins = {k: np.asarray(v) for k, v in ref.setup_inputs().items()}
    got = kernel(**ins)
    exp = np.asarray(ref.reference(**ref.setup_inputs()))
    err = np.abs(got - exp)
    denom = np.abs(exp).max()
    print(f"absmax_scaled={err.max()/denom:.3e}  mean={err.mean():.3e}")
